# revision 3
# baseline (speedup 1.0000x reference)
"""MoE grouped-MLP (Megatron GroupedMLP fwd, no gate) on 8 TRN2 NeuronCores.

Strategy: one expert per core (expert-parallel, per the sharding hint's E-axis
split).  Each core holds its expert's full w1/w2 and processes that expert's
token group; outputs are final (no partial sums), so every tensor crosses the
host<->device boundary exactly once:

  - weights ship as int8 with per-output-channel scales (4.2 MB per matrix per
    core instead of 8.4 MB bf16 / 16.8 MB fp32).  They are cast to bf16 by the
    DMA engine on load; the dequant scale folds into the PSUM evacuation step
    (activation scale operand), so no extra compute pass touches the weights.
  - x / out ship as bf16, tight to max(tokens_per_expert) columns.

All matmuls run transposed (fc1^T = w1^T @ x^T, out^T = w2^T @ act^T) so both
weight operands load in their natural [K, M] layouts: no on-device transposes.
PSUM accumulates fp32; gelu(s1 * psum) and s2 * psum run on the scalar engine
with the per-partition scale vectors.
"""

import math
from contextlib import ExitStack

import ml_dtypes
import numpy as np

import concourse.bass as bass
import concourse.mybir as mybir
import concourse.tile as tile
from concourse import bacc
from concourse.bass_utils import run_bass_kernel_spmd

NTILE = 512  # token tile (moving-operand free dim; one fp32 PSUM bank)
BF16 = mybir.dt.bfloat16
F32 = mybir.dt.float32
I8 = mybir.dt.int8
NP_BF16 = ml_dtypes.bfloat16

_NC_CACHE = {}


def _token_tiles(width):
    """Split width into [512, 512, ..., rem] matmul free-dim tiles."""
    nt, rem = divmod(width, NTILE)
    return [NTILE] * nt + ([rem] if rem else [])


def _build(width, h, f):
    """One core's program: full-FFN expert MLP over `width` token columns.

    width: token columns per core (= max tokens_per_expert, unpadded tiles).
    h: hidden size.  f: ffn size.
    """
    key = (width, h, f)
    if key in _NC_CACHE:
        return _NC_CACHE[key]

    kh = h // 128  # fc1 contraction tiles (8)
    kf = f // 128  # fc2 contraction tiles (32)
    m1 = f // 128  # fc1 output partition tiles (32)
    m2 = h // 128  # fc2 output partition tiles (8)
    tiles = _token_tiles(width)

    nc = bacc.Bacc()
    xq = nc.dram_tensor("xq", [128, kh, width], BF16, kind="ExternalInput")
    w1q = nc.dram_tensor("w1q", [128, kh, f], I8, kind="ExternalInput")
    w2q = nc.dram_tensor("w2q", [128, kf, h], I8, kind="ExternalInput")
    s1d = nc.dram_tensor("s1d", [128, m1], F32, kind="ExternalInput")
    s2d = nc.dram_tensor("s2d", [128, m2], F32, kind="ExternalInput")
    outq = nc.dram_tensor("outq", [128, m2, width], BF16, kind="ExternalOutput")

    with tile.TileContext(nc) as tc, ExitStack() as ctx:
        wpool = ctx.enter_context(tc.tile_pool(name="w", bufs=1))
        apool = ctx.enter_context(tc.tile_pool(name="act", bufs=1))
        opool = ctx.enter_context(tc.tile_pool(name="out", bufs=1))
        ps1 = ctx.enter_context(tc.tile_pool(name="ps1", bufs=4, space="PSUM"))
        ps2 = ctx.enter_context(tc.tile_pool(name="ps2", bufs=4, space="PSUM"))

        w1_sb = wpool.tile([128, kh, f], BF16, name="w1")
        w2_sb = wpool.tile([128, kf, h], BF16, name="w2")
        x_sb = wpool.tile([128, kh, width], BF16, name="x")
        s1_sb = wpool.tile([128, m1], F32, name="s1")
        s2_sb = wpool.tile([128, m2], F32, name="s2")

        # int8 -> bf16 cast happens inside the DMA (SWDGE-only feature).
        nc.gpsimd.dma_start(out=w1_sb, in_=w1q[:, :, :])
        nc.gpsimd.dma_start(out=w2_sb, in_=w2q[:, :, :])
        nc.sync.dma_start(out=x_sb, in_=xq[:, :, :])
        nc.sync.dma_start(out=s1_sb, in_=s1d[:, :])
        nc.sync.dma_start(out=s2_sb, in_=s2d[:, :])

        col = 0
        for n, w in enumerate(tiles):
            acts = []
            for m in range(m1):
                ps = ps1.tile([128, NTILE], F32, name="fc1ps", tag="fc1ps")
                for k in range(kh):
                    nc.tensor.matmul(
                        ps[:, :w],
                        w1_sb[:, k, 128 * m : 128 * (m + 1)],
                        x_sb[:, k, col : col + w],
                        start=(k == 0),
                        stop=(k == kh - 1),
                    )
                a = apool.tile([128, NTILE], BF16, name=f"a{m}", tag=f"a{m}")
                nc.scalar.activation(
                    a[:, :w],
                    ps[:, :w],
                    mybir.ActivationFunctionType.Gelu,
                    scale=s1_sb[:, m : m + 1],
                )
                acts.append(a)

            ostage = opool.tile([128, m2, NTILE], BF16, name="ostage", tag="ostage")
            for m in range(m2):
                ps = ps2.tile([128, NTILE], F32, name="fc2ps", tag="fc2ps")
                for k in range(kf):
                    nc.tensor.matmul(
                        ps[:, :w],
                        w2_sb[:, k, 128 * m : 128 * (m + 1)],
                        acts[k][:, :w],
                        start=(k == 0),
                        stop=(k == kf - 1),
                    )
                nc.scalar.activation(
                    ostage[:, m, :w],
                    ps[:, :w],
                    mybir.ActivationFunctionType.Copy,
                    scale=s2_sb[:, m : m + 1],
                )
            nc.gpsimd.dma_start(
                out=outq[:, :, col : col + w], in_=ostage[:, :, :w]
            )
            col += w

    nc.compile()
    _NC_CACHE[key] = nc
    return nc


def _quant_cols(w):
    """Symmetric per-output-channel int8: w ~= q * s with s = colmax/127."""
    s = np.abs(w).max(axis=0) / 127.0
    s = np.where(s == 0, 1.0, s).astype(np.float32)
    q = np.clip(np.rint(w / s), -127, 127).astype(np.int8)
    return q, s


def _part_major(a, chunks):
    """[chunks*128, N] -> [128, chunks, N] with [p, i, :] = a[128*i + p, :]."""
    n = a.shape[1]
    return np.ascontiguousarray(a.reshape(chunks, 128, n).transpose(1, 0, 2))


def prepare(dispatched_input, tokens_per_expert, w1, w2):
    """Build (nc, in_maps, gather) for the expert-per-core SPMD program."""
    t_tot, h = dispatched_input.shape
    e, _, f = w1.shape
    kh, kf, m1, m2 = h // 128, f // 128, f // 128, h // 128
    tpe = np.asarray(tokens_per_expert, dtype=np.int64)
    offs = np.concatenate([[0], np.cumsum(tpe)])
    width = max(int(tpe.max()), 1)

    nc = _build(width, h, f)

    x_bf = np.asarray(dispatched_input).astype(NP_BF16)
    in_maps = []
    for ei in range(e):
        t = int(tpe[ei])
        xT = np.zeros((h, width), dtype=NP_BF16)
        xT[:, :t] = x_bf[offs[ei] : offs[ei] + t].T
        q1, s1 = _quant_cols(np.asarray(w1[ei], dtype=np.float32))
        q2, s2 = _quant_cols(np.asarray(w2[ei], dtype=np.float32))
        in_maps.append(
            {
                "xq": _part_major(xT, kh),
                "w1q": _part_major(q1, kh),
                "w2q": _part_major(q2, kf),
                "s1d": np.ascontiguousarray(s1.reshape(m1, 128).T),
                "s2d": np.ascontiguousarray(s2.reshape(m2, 128).T),
            }
        )

    def gather(per_core_out):
        out = np.empty((t_tot, h), dtype=np.float32)
        for ei in range(e):
            t = int(tpe[ei])
            # outq [128, m2, width] -> out^T [h, width]
            oT = (
                per_core_out[ei]
                .transpose(1, 0, 2)
                .reshape(h, width)
                .astype(np.float32)
            )
            out[offs[ei] : offs[ei] + t] = oT[:, :t].T
        return out

    return nc, in_maps, gather


def kernel(dispatched_input, tokens_per_expert, w1, w2, _spmd_kwargs=None):
    nc, in_maps, gather = prepare(dispatched_input, tokens_per_expert, w1, w2)
    res = run_bass_kernel_spmd(
        nc, in_maps, core_ids=list(range(8)), **(_spmd_kwargs or {})
    )
    global LAST_RESULT
    LAST_RESULT = res
    return gather([r["outq"] for r in res.results])


# revision 4
# speedup vs baseline: 1.0239x; 1.0239x over previous
"""MoE grouped-MLP (Megatron GroupedMLP fwd, no gate) on 8 TRN2 NeuronCores.

Strategy: one expert per core (expert-parallel, per the sharding hint's E-axis
split).  Each core holds its expert's full w1/w2 and processes that expert's
token group; outputs are final (no partial sums), so every tensor crosses the
host<->device boundary exactly once:

  - weights ship as int8 with per-output-channel scales (4.2 MB per matrix per
    core instead of 8.4 MB bf16 / 16.8 MB fp32).  They are cast to bf16 by the
    DMA engine on load; the dequant scale folds into the PSUM evacuation step
    (activation scale operand), so no extra compute pass touches the weights.
  - x / out ship as bf16, tight to max(tokens_per_expert) columns.

All matmuls run transposed (fc1^T = w1^T @ x^T, out^T = w2^T @ act^T) so both
weight operands load in their natural [K, M] layouts: no on-device transposes.
PSUM accumulates fp32; gelu(s1 * psum) and s2 * psum run on the scalar engine
with the per-partition scale vectors.
"""

import math
from contextlib import ExitStack

import ml_dtypes
import numpy as np

import concourse.bass as bass
import concourse.mybir as mybir
import concourse.tile as tile
from concourse import bacc
from concourse.bass_utils import run_bass_kernel_spmd

NTILE = 512  # token tile (moving-operand free dim; one fp32 PSUM bank)
BF16 = mybir.dt.bfloat16
F32 = mybir.dt.float32
I8 = mybir.dt.int8
NP_BF16 = ml_dtypes.bfloat16

_NC_CACHE = {}


def _token_tiles(width):
    """Split width into [512, 512, ..., rem] matmul free-dim tiles."""
    nt, rem = divmod(width, NTILE)
    return [NTILE] * nt + ([rem] if rem else [])


def _build(width, h, f):
    """One core's program: full-FFN expert MLP over `width` token columns.

    width: token columns per core (= max tokens_per_expert, unpadded tiles).
    h: hidden size.  f: ffn size.
    """
    key = (width, h, f)
    if key in _NC_CACHE:
        return _NC_CACHE[key]

    kh = h // 128  # fc1 contraction tiles (8)
    kf = f // 128  # fc2 contraction tiles (32)
    m1 = f // 128  # fc1 output partition tiles (32)
    m2 = h // 128  # fc2 output partition tiles (8)
    tiles = _token_tiles(width)

    nc = bacc.Bacc()
    xq = nc.dram_tensor("xq", [128, kh, width], BF16, kind="ExternalInput")
    w1q = nc.dram_tensor("w1q", [128, kh, f], I8, kind="ExternalInput")
    w2q = nc.dram_tensor("w2q", [128, kf, h], I8, kind="ExternalInput")
    s1d = nc.dram_tensor("s1d", [128, m1], F32, kind="ExternalInput")
    s2d = nc.dram_tensor("s2d", [128, m2], F32, kind="ExternalInput")
    outq = nc.dram_tensor("outq", [128, m2, width], BF16, kind="ExternalOutput")

    with tile.TileContext(nc) as tc, ExitStack() as ctx:
        wpool = ctx.enter_context(tc.tile_pool(name="w", bufs=1))
        apool = ctx.enter_context(tc.tile_pool(name="act", bufs=1))
        opool = ctx.enter_context(tc.tile_pool(name="out", bufs=1))
        ps1 = ctx.enter_context(tc.tile_pool(name="ps1", bufs=4, space="PSUM"))
        ps2 = ctx.enter_context(tc.tile_pool(name="ps2", bufs=4, space="PSUM"))

        w1_sb = wpool.tile([128, kh, f], BF16, name="w1")
        w2_sb = wpool.tile([128, kf, h], BF16, name="w2")
        x_sb = wpool.tile([128, kh, width], BF16, name="x")
        s1_sb = wpool.tile([128, m1], F32, name="s1")
        s2_sb = wpool.tile([128, m2], F32, name="s2")

        # Loads are chunked along the contraction dim and issued in first-use
        # order so the first fc1 matmul only waits for chunk 0, not the full
        # 21 MB.  int8 -> bf16 cast happens inside the DMA (SWDGE-only).
        nc.sync.dma_start(out=s1_sb, in_=s1d[:, :])
        nc.sync.dma_start(out=s2_sb, in_=s2d[:, :])
        for k in range(kh):
            nc.gpsimd.dma_start(out=w1_sb[:, k, :], in_=w1q[:, k, :])
            nc.sync.dma_start(out=x_sb[:, k, :], in_=xq[:, k, :])
        for k in range(kf):
            nc.gpsimd.dma_start(out=w2_sb[:, k, :], in_=w2q[:, k, :])

        col = 0
        for n, w in enumerate(tiles):
            acts = []
            for m in range(m1):
                ps = ps1.tile([128, NTILE], F32, name="fc1ps", tag="fc1ps")
                for k in range(kh):
                    nc.tensor.matmul(
                        ps[:, :w],
                        w1_sb[:, k, 128 * m : 128 * (m + 1)],
                        x_sb[:, k, col : col + w],
                        start=(k == 0),
                        stop=(k == kh - 1),
                    )
                a = apool.tile([128, NTILE], BF16, name=f"a{m}", tag=f"a{m}")
                nc.scalar.activation(
                    a[:, :w],
                    ps[:, :w],
                    mybir.ActivationFunctionType.Gelu,
                    scale=s1_sb[:, m : m + 1],
                )
                acts.append(a)

            ostage = opool.tile([128, m2, NTILE], BF16, name="ostage", tag="ostage")
            for m in range(m2):
                ps = ps2.tile([128, NTILE], F32, name="fc2ps", tag="fc2ps")
                for k in range(kf):
                    nc.tensor.matmul(
                        ps[:, :w],
                        w2_sb[:, k, 128 * m : 128 * (m + 1)],
                        acts[k][:, :w],
                        start=(k == 0),
                        stop=(k == kf - 1),
                    )
                nc.scalar.activation(
                    ostage[:, m, :w],
                    ps[:, :w],
                    mybir.ActivationFunctionType.Copy,
                    scale=s2_sb[:, m : m + 1],
                )
            nc.gpsimd.dma_start(
                out=outq[:, :, col : col + w], in_=ostage[:, :, :w]
            )
            col += w

    nc.compile()
    _NC_CACHE[key] = nc
    return nc


def _quant_cols(w):
    """Symmetric per-output-channel int8: w ~= q * s with s = colmax/127."""
    s = np.abs(w).max(axis=0) / 127.0
    s = np.where(s == 0, 1.0, s).astype(np.float32)
    q = np.clip(np.rint(w / s), -127, 127).astype(np.int8)
    return q, s


def _part_major(a, chunks):
    """[chunks*128, N] -> [128, chunks, N] with [p, i, :] = a[128*i + p, :]."""
    n = a.shape[1]
    return np.ascontiguousarray(a.reshape(chunks, 128, n).transpose(1, 0, 2))


def prepare(dispatched_input, tokens_per_expert, w1, w2):
    """Build (nc, in_maps, gather) for the expert-per-core SPMD program."""
    t_tot, h = dispatched_input.shape
    e, _, f = w1.shape
    kh, kf, m1, m2 = h // 128, f // 128, f // 128, h // 128
    tpe = np.asarray(tokens_per_expert, dtype=np.int64)
    offs = np.concatenate([[0], np.cumsum(tpe)])
    width = max(int(tpe.max()), 1)

    nc = _build(width, h, f)

    x_bf = np.asarray(dispatched_input).astype(NP_BF16)
    in_maps = []
    for ei in range(e):
        t = int(tpe[ei])
        xT = np.zeros((h, width), dtype=NP_BF16)
        xT[:, :t] = x_bf[offs[ei] : offs[ei] + t].T
        q1, s1 = _quant_cols(np.asarray(w1[ei], dtype=np.float32))
        q2, s2 = _quant_cols(np.asarray(w2[ei], dtype=np.float32))
        in_maps.append(
            {
                "xq": _part_major(xT, kh),
                "w1q": _part_major(q1, kh),
                "w2q": _part_major(q2, kf),
                "s1d": np.ascontiguousarray(s1.reshape(m1, 128).T),
                "s2d": np.ascontiguousarray(s2.reshape(m2, 128).T),
            }
        )

    def gather(per_core_out):
        out = np.empty((t_tot, h), dtype=np.float32)
        for ei in range(e):
            t = int(tpe[ei])
            # outq [128, m2, width] -> out^T [h, width]
            oT = (
                per_core_out[ei]
                .transpose(1, 0, 2)
                .reshape(h, width)
                .astype(np.float32)
            )
            out[offs[ei] : offs[ei] + t] = oT[:, :t].T
        return out

    return nc, in_maps, gather


def kernel(dispatched_input, tokens_per_expert, w1, w2, _spmd_kwargs=None):
    nc, in_maps, gather = prepare(dispatched_input, tokens_per_expert, w1, w2)
    res = run_bass_kernel_spmd(
        nc, in_maps, core_ids=list(range(8)), **(_spmd_kwargs or {})
    )
    global LAST_RESULT
    LAST_RESULT = res
    return gather([r["outq"] for r in res.results])
